# revision 14
# baseline (speedup 1.0000x reference)
# Trainium2 Bass kernel for nn_CausalSelfAttention_13022340841799.
#
# Problem (hardcoded shapes): B=2, L=4096, D=512, 8 heads of dim 64.
#   qkv = x @ w_in + b_in; prefix-causal attention (PREFIX=1: tril mask with
#   column 0 disallowed for rows >= 1); out = attn_out @ w_out + b_out.
#
# Sharding: 8 cores = 2 batches x 4 head-pairs. Core c handles batch c//4 and
# heads {2*(c%4), 2*(c%4)+1}. Each core computes a partial [L, D] output
# (its heads' contribution through w_out); the host sums the 4 partials per
# batch and adds b_out.
#
# Device design (all bf16 compute, f32 PSUM):
#  - Host pre-transposes x to xT [128, 4, L] bf16 (no on-device transposes).
#  - qT/kT [feat, L] from wq/wk lhsT matmuls; v natural [L, feat] + ones col.
#  - S^T tiles [128 keys, 512 queries] per head computed as ROW-TILED pairs:
#    h0 on PE rows 0-63 (tile_position (0,0)), h1 on rows 64-127 ((64,0)) --
#    the two K=64 matmuls run concurrently in the array, out to 2 psum banks.
#  - Diagonal tiles only compute queries >= 128*d (the skipped region is
#    provably never read by PV). Their mask collapses to k <= q_local on the
#    first 128 query columns -> one shared [128,128] mask tile.
#  - Prefix (key-0) masking is folded into the exp bias (-80 at partition 0).
#  - exp is split between ACT (native Exp) and DVE (fast-exp bit trick:
#    P_bf16_bits = S*184.665 + 16251 as int16, bitcast to bf16).
#  - PV transposed: O^T_aug [65, 512] += v_aug_j^T @ P_j; row 64 accumulates
#    the softmax denominator via the ones column of v_aug.
#  - Normalize: DVE reciprocal of denom row -> PE ones-broadcast matmul to
#    spread 1/denom across partitions -> DVE mult -> O^T [64, 2, L] bf16.
#  - out partial = sum_h O_h^T.T @ wo_h in PSUM, DMA'd f32 PSUM->DRAM.
#  - Emission is software-pipelined per 512-row chunk so PE stays dense.

import numpy as np
import ml_dtypes

import concourse.bass as bass
import concourse.mybir as mybir
import concourse.tile as tile
from concourse import bacc
from concourse.bass_utils import run_bass_kernel_spmd

F32 = mybir.dt.float32
BF16 = mybir.dt.bfloat16
I16 = mybir.dt.int16

B, L, D = 2, 4096, 512
H, HD = 8, 64
HPC = 2                  # heads per core
CD = HPC * HD            # 128 per-core qkv feature columns
NCORES = 8
SCALE = 1.0 / 8.0        # 1/sqrt(64)
NRC = L // 512           # 8 row chunks
KC = D // 128            # 4 contraction chunks
MASK_NEG = -80.0         # pre-exp additive mask (exp(-80+s) ~ 0)
# fast-exp: bf16_bits(e^x) ~ x * (2^7/ln2) + (16256 - 5.46 + 0.5)
FE_A = 128.0 / float(np.log(2.0))
FE_B = 16251.04
LAG = 2                  # S-pair -> PV software pipeline distance


def _route_dve(r, j):
    """Which exp tiles go to DVE (fast-exp) vs ACT (native exp).

    Near-alternation at ~43% DVE: short same-engine runs so neither
    engine builds a backlog that stalls the PE's S-tile psum rotation
    (S(j) claim waits on exp(j-2))."""
    return (j % 7) in (1, 3, 5)


def build_kernel(dbg=False):
    nc = bacc.Bacc(trn_type="TRN2", target_bir_lowering=False)

    xt_d = nc.declare_dram_parameter("xt", [128, KC, L], BF16, isOutput=False)
    wq_d = nc.declare_dram_parameter("wq", [128, KC, CD], BF16, isOutput=False)
    wk_d = nc.declare_dram_parameter("wk", [128, KC, CD], BF16, isOutput=False)
    wv_d = nc.declare_dram_parameter("wv", [128, KC, CD], BF16, isOutput=False)
    wo_d = nc.declare_dram_parameter("wo", [64, HPC, D], BF16, isOutput=False)
    bq_d = nc.declare_dram_parameter("bq", [CD, 1], F32, isOutput=False)
    bk_d = nc.declare_dram_parameter("bk", [CD, 1], F32, isOutput=False)
    bv_d = nc.declare_dram_parameter("bv", [1, CD], BF16, isOutput=False)
    out_d = nc.declare_dram_parameter("out", [L, D], BF16, isOutput=True)
    if dbg:
        otd = nc.declare_dram_parameter("ot_dbg", [64, HPC, L], BF16, isOutput=True)
        ord_ = nc.declare_dram_parameter("or_dbg", [64, HPC, L], BF16, isOutput=True)
        rcd = nc.declare_dram_parameter("rc_dbg", [64, HPC, L], F32, isOutput=True)

    with tile.TileContext(nc) as tc:
        with (
            tc.tile_pool(name="const", bufs=1) as const,
            tc.tile_pool(name="ppool", bufs=4) as ppool,
            tc.tile_pool(name="work", bufs=2) as work,
            tc.tile_pool(name="psS", bufs=2, space="PSUM") as psS,
            tc.tile_pool(name="psPV", bufs=2, space="PSUM") as psPV,
            tc.tile_pool(name="psQK", bufs=1, space="PSUM") as psQK,
            tc.tile_pool(name="psVO", bufs=1, space="PSUM") as psVO,
            tc.tile_pool(name="dramp", bufs=2, space="DRAM") as dramp,
        ):
            # ---- persistent SBUF tensors
            xT = const.tile([128, KC, L], BF16, name="xT")
            qT = const.tile([128, L], BF16, name="qT")
            kT = const.tile([128, L], BF16, name="kT")
            v_aug = [
                const.tile([128, L // 128, 65], BF16, name=f"vaug{h}")
                for h in range(HPC)
            ]
            OT = const.tile([64, HPC, L], BF16, name="OT")

            wq_s = const.tile([128, KC, CD], BF16, name="wq_s")
            wk_s = const.tile([128, KC, CD], BF16, name="wk_s")
            wv_s = const.tile([128, KC, CD], BF16, name="wv_s")
            wo_s = const.tile([64, HPC, D], BF16, name="wo_s")
            bq_s = const.tile([CD, 1], F32, name="bq_s")
            bk_s = const.tile([CD, 1], F32, name="bk_s")
            bv_s = const.tile([1, CD], BF16, name="bv_s")

            nc.sync.dma_start(wq_s, wq_d[:, :, :])
            nc.sync.dma_start(wk_s, wk_d[:, :, :])
            nc.sync.dma_start(wv_s, wv_d[:, :, :])
            nc.sync.dma_start(wo_s, wo_d[:, :, :])
            nc.sync.dma_start(bq_s, bq_d[:, :])
            nc.sync.dma_start(bk_s, bk_d[:, :])
            nc.sync.dma_start(bv_s, bv_d[:, :])
            for r in range(NRC):
                cs = slice(r * 512, (r + 1) * 512)
                nc.sync.dma_start(xT[:, :, cs], xt_d[:, :, cs])

            # ---- constants
            ones128 = const.tile([1, 128], BF16, name="ones128")
            nc.gpsimd.memset(ones128, 1.0)
            for h in range(HPC):
                nc.gpsimd.memset(v_aug[h][:, :, 64:65], 1.0)

            # causal mask tile: M[k, q] = 1 if k <= q else 0
            Mc = const.tile([128, 128], BF16, name="Mc")
            nc.gpsimd.memset(Mc, 1.0)
            nc.gpsimd.affine_select(
                out=Mc, in_=Mc, compare_op=mybir.AluOpType.is_ge, fill=0.0,
                base=0, channel_multiplier=-1, pattern=[[1, 128]],
            )
            # exp bias vectors (per-partition): key-0 prefix masking
            b0_act = const.tile([128, 1], F32, name="b0_act")
            nc.gpsimd.memset(b0_act, 0.0)
            nc.gpsimd.memset(b0_act[0:1, :], MASK_NEG)
            b_dve = const.tile([128, 1], F32, name="b_dve")
            nc.gpsimd.memset(b_dve, FE_B)
            b0_dve = const.tile([128, 1], F32, name="b0_dve")
            nc.gpsimd.memset(b0_dve, FE_B)
            nc.gpsimd.memset(b0_dve[0:1, :], FE_B + MASK_NEG * FE_A)

            # ---- per-chunk emission helpers
            def emit_qkv(r):
                cs = slice(r * 512, (r + 1) * 512)
                pq = psQK.tile([128, 512], F32, tag="qk")
                for d in range(KC):
                    nc.tensor.matmul(
                        pq, lhsT=wq_s[:, d, :], rhs=xT[:, d, cs],
                        start=(d == 0), stop=(d == KC - 1),
                    )
                nc.scalar.activation(
                    qT[:, cs], pq, mybir.ActivationFunctionType.Identity,
                    bias=bq_s, scale=SCALE,
                )
                pk = psQK.tile([128, 512], F32, tag="qk")
                for d in range(KC):
                    nc.tensor.matmul(
                        pk, lhsT=wk_s[:, d, :], rhs=xT[:, d, cs],
                        start=(d == 0), stop=(d == KC - 1),
                    )
                nc.scalar.activation(
                    kT[:, cs], pk, mybir.ActivationFunctionType.Identity,
                    bias=bk_s, scale=1.0,
                )
                pv = psVO.tile([128, 512], F32, tag="vo")
                for rb in range(4):
                    rs = slice((4 * r + rb) * 128, (4 * r + rb + 1) * 128)
                    ps = pv[:, rb * 128:(rb + 1) * 128]
                    for d in range(KC):
                        nc.tensor.matmul(
                            ps, lhsT=xT[:, d, rs], rhs=wv_s[:, d, :],
                            start=(d == 0), stop=False,
                        )
                    nc.tensor.matmul(
                        ps, lhsT=ones128, rhs=bv_s, start=False, stop=True,
                    )
                pvv = pv.rearrange("p (g c) -> p g c", c=128)
                for h in range(HPC):
                    nc.vector.tensor_copy(
                        v_aug[h][:, 4 * r:4 * r + 4, 0:64],
                        pvv[:, :, h * 64:(h + 1) * 64],
                    )

            def emit_S(r, j):
                d = j - 4 * r
                qoff = 128 * d if d >= 0 else 0
                w = 512 - qoff
                sp = psS.tile([128, 2, 512], F32, tag="sp")
                for h in range(HPC):
                    hs = slice(h * 64, (h + 1) * 64)
                    nc.tensor.matmul(
                        sp[:, h, 0:w],
                        lhsT=kT[hs, j * 128:(j + 1) * 128],
                        rhs=qT[hs, r * 512 + qoff:(r + 1) * 512],
                        start=True, stop=True,
                        tile_position=(64 * h, 0),
                    )
                # exp -> P bf16 (both heads in one instruction)
                pt = ppool.tile([128, 2, 512], BF16, tag="p")
                if _route_dve(r, j):
                    bias = b0_dve if j == 0 else b_dve
                    nc.vector.tensor_scalar(
                        out=pt.bitcast(I16)[:, :, 0:w], in0=sp[:, :, 0:w],
                        scalar1=FE_A, scalar2=bias,
                        op0=mybir.AluOpType.mult, op1=mybir.AluOpType.add,
                    )
                else:
                    bias = b0_act if j == 0 else 0.0
                    nc.scalar.activation(
                        pt[:, :, 0:w], sp[:, :, 0:w],
                        mybir.ActivationFunctionType.Exp,
                        bias=bias, scale=1.0,
                    )
                if d >= 0:
                    # diagonal: mask first 128 query cols with k<=q pattern
                    mb = bass.AP(
                        tensor=Mc.tensor, offset=Mc.offset,
                        ap=[list(Mc.ap[0]), [0, 2], [1, 128]],
                    )
                    nc.gpsimd.tensor_tensor(
                        pt[:, :, 0:128], pt[:, :, 0:128], mb,
                        mybir.AluOpType.mult,
                    )
                if r == 0 and j == 0:
                    # query 0 attends only key 0: force P[0, 0] = 1
                    nc.vector.memset(pt[0:1, :, 0:1], 1.0)
                return pt

            def emit_PV(r, j, pv_ts, p_ts, nj):
                d = j - 4 * r
                qoff = 128 * d if d >= 0 else 0
                for h in range(HPC):
                    nc.tensor.matmul(
                        pv_ts[h][:, qoff:512],
                        lhsT=v_aug[h][:, j, :],
                        rhs=p_ts[j][:, h, 0:512 - qoff],
                        start=(j == 0), stop=(j == nj - 1),
                    )

            def emit_recip(r, pv_ts):
                # Evacuate O_raw psum->SBUF and start the 1/denominator
                # broadcast (pipelined DRAM bounce, consumed ~5us later).
                # Must be emitted before the next chunk's PV claims so the
                # psum WAR deps see every reader.
                dn = work.tile([65, 2, 512], F32, tag="dn")
                o_raw = work.tile([64, 2, 512], BF16, tag="o_raw")
                for h in range(HPC):
                    # psum reads must stay partition-aligned (engines cannot
                    # shift partitions) and on ACT/DVE (gpsimd has no psum
                    # access)
                    nc.scalar.activation(
                        dn[64:65, h, :], pv_ts[h][64:65, :],
                        mybir.ActivationFunctionType.Copy)
                    nc.vector.tensor_copy(o_raw[:, h, :], pv_ts[h][0:64, :])
                # broadcast raw denominators to partitions 0-63 via DRAM
                # bounce, then invert there (reciprocal_approx_fast only
                # works at base partition 0)
                scr = dramp.tile([1, 2, 512], F32, tag="scr")
                nc.sync.dma_start(out=scr, in_=dn[64:65, :, :])
                dnb = work.tile([64, 2, 512], F32, tag="dnb")
                s = scr[0:1, :, :]
                src_b = bass.AP(
                    tensor=s.tensor, offset=s.offset,
                    ap=[[0, 64]] + [list(p) for p in s.ap[1:]],
                )
                nc.sync.dma_start(out=dnb, in_=src_b)
                rc_b = work.tile([64, 2, 512], F32, tag="rc_b")
                nc.vector.reciprocal_approx_fast(out=rc_b, in_=dnb)
                return o_raw, rc_b

            def emit_norm(r, o_raw, rc_b):
                cs = slice(r * 512, (r + 1) * 512)
                for h in range(HPC):
                    nc.vector.tensor_tensor(
                        OT[:, h, cs], o_raw[:, h, :], rc_b[:, h, :],
                        mybir.AluOpType.mult,
                    )
                if dbg:
                    nc.sync.dma_start(ord_[:, :, cs], o_raw)
                    nc.sync.dma_start(rcd[:, :, cs], rc_b)

            def emit_outproj(r, blk):
                bs = slice((4 * r + blk) * 128, (4 * r + blk + 1) * 128)
                op = psVO.tile([128, 512], F32, tag="vo")
                for h in range(HPC):
                    nc.tensor.matmul(
                        op, lhsT=OT[:, h, bs], rhs=wo_s[:, h, :],
                        start=(h == 0), stop=(h == HPC - 1),
                    )
                ost = work.tile([128, 512], BF16, tag="ost")
                if blk % 2 == 0:
                    nc.scalar.activation(
                        ost, op, mybir.ActivationFunctionType.Copy)
                else:
                    nc.vector.tensor_copy(ost, op)
                nc.sync.dma_start(out_d[bs, :], ost)

            # ---- main pipeline
            emit_qkv(0)
            prev = None           # (r-1)'s pv tiles, for pipelined normalize
            for r in range(NRC):
                nj = 4 * r + 4
                nstep = nj + LAG
                op_stride = 1 if nstep < 13 else 2
                op_slots = [5 + i * op_stride for i in range(4)]
                pv_ts = [
                    psPV.tile([65, 512], F32, tag="pv", name=f"pv{h}")
                    for h in range(HPC)
                ]
                p_ts = {}
                for step in range(nstep):
                    if step < nj:
                        p_ts[step] = emit_S(r, step)
                    if step == 1 and r + 1 < NRC:
                        emit_qkv(r + 1)
                    if step == 3 and prev is not None:
                        emit_norm(r - 1, *prev)
                    if prev is not None and step in op_slots:
                        emit_outproj(r - 1, op_slots.index(step))
                    if step >= LAG:
                        j = step - LAG
                        emit_PV(r, j, pv_ts, p_ts, nj)
                        p_ts.pop(j)
                prev = emit_recip(r, pv_ts)
            emit_norm(NRC - 1, *prev)
            for blk in range(4):
                emit_outproj(NRC - 1, blk)
            if dbg:
                nc.sync.dma_start(otd[:, :, :], OT)

    nc.finalize()
    return nc


def _shard_inputs(x, w_in, b_in, w_out):
    """Per-core input maps: core c -> batch c//4, heads pair c%4."""
    bf16 = ml_dtypes.bfloat16
    in_maps = []
    for c in range(NCORES):
        b = c // 4
        hp = c % 4
        cs = slice(hp * CD, hp * CD + CD)

        xt = np.ascontiguousarray(x[b].T)          # [D, L] f32
        xt = np.ascontiguousarray(
            xt.reshape(KC, 128, L).transpose(1, 0, 2), dtype=bf16)

        def wslice(lo):
            w = w_in[:, lo:lo + D][:, cs]           # [D, CD]
            return np.ascontiguousarray(
                w.reshape(KC, 128, CD).transpose(1, 0, 2), dtype=bf16)

        wo = w_out[cs, :]                           # [CD, D]
        wo = np.ascontiguousarray(
            wo.reshape(HPC, 64, D).transpose(1, 0, 2), dtype=bf16)

        in_maps.append({
            "xt": xt,
            "wq": wslice(0),
            "wk": wslice(D),
            "wv": wslice(2 * D),
            "wo": wo,
            "bq": np.ascontiguousarray(
                (b_in[0:D][cs] * SCALE).reshape(CD, 1), dtype=np.float32),
            "bk": np.ascontiguousarray(
                b_in[D:2 * D][cs].reshape(CD, 1), dtype=np.float32),
            "bv": np.ascontiguousarray(
                b_in[2 * D:3 * D][cs].reshape(1, CD), dtype=bf16),
        })
    return in_maps


_NC_CACHE = None


def _get_nc():
    global _NC_CACHE
    if _NC_CACHE is None:
        _NC_CACHE = build_kernel()
    return _NC_CACHE


def run(x, w_in, b_in, w_out, b_out, trace=False, **spmd_kwargs):
    x = np.asarray(x, dtype=np.float32)
    w_in = np.asarray(w_in, dtype=np.float32)
    b_in = np.asarray(b_in, dtype=np.float32)
    w_out = np.asarray(w_out, dtype=np.float32)
    b_out = np.asarray(b_out, dtype=np.float32)

    nc = _get_nc()
    in_maps = _shard_inputs(x, w_in, b_in, w_out)
    res = run_bass_kernel_spmd(
        nc, in_maps, core_ids=list(range(NCORES)), trace=trace, **spmd_kwargs
    )
    out = np.zeros((B, L, D), dtype=np.float32)
    for c in range(NCORES):
        out[c // 4] += np.asarray(res.results[c]["out"], dtype=np.float32)
    out += b_out[None, None, :]
    return out, res


def kernel(x, w_in, b_in, w_out, b_out):
    out, _ = run(x, w_in, b_in, w_out, b_out, trace=False)
    return out
